# revision 8
# baseline (speedup 1.0000x reference)
"""Trainium2 Bass kernel for one attention-LSTM decoder step.

dims: B=512, S=64, H=1024, E=256, V=128, L=2, sharded data-parallel over
batch across 8 NeuronCores (64 batches/core). All matmuls run in fp16 with
fp32 PSUM accumulation; elementwise/softmax math in fp32.
"""

import sys

if "/opt/trn_rl_repo" not in sys.path:
    sys.path.insert(0, "/opt/trn_rl_repo")

import numpy as np

import concourse.bacc as bacc
import concourse.mybir as mybir
import concourse.tile as tile
from concourse.bass_utils import run_bass_kernel_spmd

B, S, H, E, V = 512, 64, 1024, 256, 128
NCORES = 8
BC = B // NCORES          # 64 batches per core
TOK = BC * S              # 4096 tokens per core
F = E + H                 # 1280 LSTM input features
G4 = 4 * H                # 4096 gate rows
F16 = mybir.dt.float16
F32 = mybir.dt.float32
AF = mybir.ActivationFunctionType
ALU = mybir.AluOpType
AX = mybir.AxisListType

_COMPILED = [None]


def _build():
    nc = bacc.Bacc("TRN2", target_bir_lowering=False, debug=False,
                   num_devices=NCORES)

    # ---- DRAM I/O ----
    d_encT = nc.dram_tensor("encT", [H, TOK], F16, kind="ExternalInput")
    d_encS = nc.dram_tensor("encS", [TOK, H], F16, kind="ExternalInput")
    d_U = nc.dram_tensor("Umat", [H, H], F16, kind="ExternalInput")
    d_W = nc.dram_tensor("Wmat", [H, H], F16, kind="ExternalInput")
    d_Vw = nc.dram_tensor("VwR", [128, 8], F16, kind="ExternalInput")
    d_hT0 = nc.dram_tensor("hT0", [128, 8 * BC], F16, kind="ExternalInput")
    d_hT1 = nc.dram_tensor("hT1", [128, 8 * BC], F16, kind="ExternalInput")
    d_ohT = nc.dram_tensor("onehotT", [V, BC], F16, kind="ExternalInput")
    d_emb = nc.dram_tensor("embW", [V, E], F16, kind="ExternalInput")
    d_oh64 = nc.dram_tensor("oh64", [BC, TOK], F16, kind="ExternalInput")
    d_I2 = nc.dram_tensor("I2", [128, 64], F32, kind="ExternalInput")
    d_I128 = nc.dram_tensor("I128", [128, 128], F16, kind="ExternalInput")
    d_ones1 = nc.dram_tensor("ones1", [1, BC], F16, kind="ExternalInput")
    d_wi0 = nc.dram_tensor("Wih0T", [F, G4], F16, kind="ExternalInput")
    d_wh0 = nc.dram_tensor("Whh0T", [H, G4], F16, kind="ExternalInput")
    d_wi1 = nc.dram_tensor("Wih1T", [H, G4], F16, kind="ExternalInput")
    d_wh1 = nc.dram_tensor("Whh1T", [H, G4], F16, kind="ExternalInput")
    d_b0 = nc.dram_tensor("bias0", [1, G4], F16, kind="ExternalInput")
    d_b1 = nc.dram_tensor("bias1", [1, G4], F16, kind="ExternalInput")
    d_c0 = nc.dram_tensor("cell0", [BC, H], F32, kind="ExternalInput")
    d_c1 = nc.dram_tensor("cell1", [BC, H], F32, kind="ExternalInput")
    d_owT = nc.dram_tensor("outWT", [H, V], F16, kind="ExternalInput")
    d_ob = nc.dram_tensor("outB", [1, V], F16, kind="ExternalInput")

    d_logits = nc.dram_tensor("logits", [BC, V], F32, kind="ExternalOutput")
    d_h0n = nc.dram_tensor("h0n", [BC, H], F32, kind="ExternalOutput")
    d_h1n = nc.dram_tensor("h1n", [BC, H], F32, kind="ExternalOutput")
    d_c0n = nc.dram_tensor("c0n", [BC, H], F32, kind="ExternalOutput")
    d_c1n = nc.dram_tensor("c1n", [BC, H], F32, kind="ExternalOutput")

    with tile.TileContext(nc) as tc:
        with (
            tc.tile_pool(name="const", bufs=1) as cpool,
            tc.tile_pool(name="encT", bufs=2) as etp,
            tc.tile_pool(name="encS", bufs=4) as esp,
            tc.tile_pool(name="tanh", bufs=2) as thp,
            tc.tile_pool(name="wls", bufs=26) as wpool,
            tc.tile_pool(name="mid", bufs=1) as mid,
            tc.tile_pool(name="psA", bufs=2, space="PSUM") as psA,
            tc.tile_pool(name="psE", bufs=2, space="PSUM") as psE,
            tc.tile_pool(name="psS", bufs=2, space="PSUM") as psS,
            tc.tile_pool(name="psT", bufs=2, space="PSUM") as psT,
        ):
            # ---------- resident constants ----------
            U_sb = [cpool.tile([128, H], F16, tag=f"U{k}", name=f"U{k}") for k in range(8)]
            for k in range(8):
                nc.sync.dma_start(U_sb[k][:], d_U[128 * k:128 * (k + 1), :])
            vw_sb = cpool.tile([128, 8], F16, tag="vw")
            nc.sync.dma_start(vw_sb[:], d_Vw[:])
            hT0_sb = cpool.tile([128, 8 * BC], F16, tag="hT0")
            nc.sync.dma_start(hT0_sb[:], d_hT0[:])
            hT1_sb = cpool.tile([128, 8 * BC], F16, tag="hT1")
            nc.sync.dma_start(hT1_sb[:], d_hT1[:])
            ohT_sb = cpool.tile([V, BC], F16, tag="ohT")
            nc.sync.dma_start(ohT_sb[:], d_ohT[:])
            emb_sb = cpool.tile([V, E], F16, tag="emb")
            nc.sync.dma_start(emb_sb[:], d_emb[:])
            I2_sb = cpool.tile([128, 64], F32, tag="I2")
            nc.sync.dma_start(I2_sb[:], d_I2[:])
            I128_sb = cpool.tile([128, 128], F16, tag="I128")
            nc.sync.dma_start(I128_sb[:], d_I128[:])
            ones1_sb = cpool.tile([1, BC], F16, tag="ones1")
            nc.sync.dma_start(ones1_sb[:], d_ones1[:])
            c0_sb = cpool.tile([BC, H], F32, tag="c0")
            nc.sync.dma_start(c0_sb[:], d_c0[:])
            c1_sb = cpool.tile([BC, H], F32, tag="c1")
            nc.sync.dma_start(c1_sb[:], d_c1[:])
            owT_sb = [cpool.tile([128, V], F16, tag=f"ow{k}", name=f"ow{k}") for k in range(8)]
            for k in range(8):
                nc.sync.dma_start(owT_sb[k][:], d_owT[128 * k:128 * (k + 1), :])
            ob_sb = cpool.tile([1, V], F16, tag="ob")
            nc.sync.dma_start(ob_sb[:], d_ob[:])

            # ---------- phase A: t2 = h_top @ W  -> [BC, H] f16 ----------
            t2_sb = mid.tile([BC, H], F16, tag="t2")
            for n2 in range(2):
                ps = psS.tile([BC, 512], F32, tag="ps64", name="t2ps")
                for k in range(8):
                    wsl = esp.tile([128, 512], F16, tag="encS", name="wsl")
                    nc.sync.dma_start(
                        wsl[:], d_W[128 * k:128 * (k + 1),
                                    512 * n2:512 * (n2 + 1)])
                    nc.tensor.matmul(
                        ps[:], hT1_sb[:, 64 * k:64 * (k + 1)], wsl[:],
                        start=(k == 0), stop=(k == 7))
                nc.vector.tensor_copy(t2_sb[:, 512 * n2:512 * (n2 + 1)], ps[:])

            # ---------- phase B: embedded^T -> xT blocks 0..1 ----------
            xT_sb = mid.tile([128, 10 * BC], F16, tag="xT")
            for et in range(2):
                ps = psT.tile([128, BC], F32, tag="pT", name="embps")
                nc.tensor.matmul(ps[:], emb_sb[:, 128 * et:128 * (et + 1)],
                                 ohT_sb[:], start=True, stop=True)
                nc.vector.tensor_copy(xT_sb[:, BC * et:BC * (et + 1)], ps[:])

            # ---------- phase C: t1 + t2 -> tanh -> scores ----------
            e_row = mid.tile([1, TOK], F16, tag="e_row")
            for n in range(8):          # token tiles (512 tokens, 8 batches)
                ets = []
                for k in range(8):
                    et = etp.tile([128, 512], F16, tag=f"encT{k}")
                    nc.sync.dma_start(
                        et[:], d_encT[128 * k:128 * (k + 1),
                                      512 * n:512 * (n + 1)])
                    ets.append(et)
                oh64_t = esp.tile([BC, 512], F16, tag="encS", name="oh64_t")
                nc.sync.dma_start(oh64_t[:], d_oh64[:, 512 * n:512 * (n + 1)])
                pe = psE.tile([1, 512], F32, tag="eps")
                for m in range(8):      # output-H tiles
                    pt = psA.tile([128, 512], F32, tag="t1ps")
                    for k in range(8):
                        nc.tensor.matmul(
                            pt[:], U_sb[k][:, 128 * m:128 * (m + 1)], ets[k][:],
                            start=(k == 0), stop=False)
                    # inject t2 broadcast over s:  lhsT=[64b,128h'] rhs=[64b,512tok]
                    nc.tensor.matmul(
                        pt[:], t2_sb[:, 128 * m:128 * (m + 1)],
                        oh64_t[:], start=False, stop=True)
                    th = thp.tile([128, 512], F16, tag="tanh")
                    nc.scalar.activation(th[:], pt[:], AF.Tanh)
                    nc.tensor.matmul(pe[:], vw_sb[:, m:m + 1], th[:],
                                     start=(m == 0), stop=(m == 7))
                nc.vector.tensor_copy(e_row[:, 512 * n:512 * (n + 1)], pe[:])

            # ---------- phase D: softmax over s (rows b, duplicated halves) --
            a2 = mid.tile([128, S], F32, tag="a2")
            src = e_row[0:1, :].rearrange("p (b s) -> p b s", b=BC)
            nc.gpsimd.dma_start(a2[0:BC, :], src)
            nc.gpsimd.dma_start(a2[BC:128, :], src)
            mx = mid.tile([128, 1], F32, tag="mx")
            nc.vector.tensor_reduce(mx[:], a2[:], axis=AX.X, op=ALU.max)
            negmx = mid.tile([128, 1], F32, tag="negmx")
            nc.vector.tensor_scalar_mul(negmx[:], mx[:], -1.0)
            p2 = mid.tile([128, S], F32, tag="p2")
            nc.scalar.activation(p2[:], a2[:], AF.Exp, bias=negmx[:])
            sm = mid.tile([128, 1], F32, tag="sm")
            nc.vector.tensor_reduce(sm[:], p2[:], axis=AX.X, op=ALU.add)
            rinv = mid.tile([128, 1], F32, tag="rinv")
            nc.vector.reciprocal(rinv[:], sm[:])
            a2w = mid.tile([128, S], F32, tag="a2w")
            nc.vector.tensor_scalar_mul(a2w[:], p2[:], rinv[:])
            # a3[p,k] = a2w[p, 2k + (p>=64)]
            a3 = mid.tile([128, 32], F32, tag="a3")
            a2w_v = a2w[:].rearrange("p (k two) -> p k two", two=2)
            nc.vector.tensor_copy(a3[0:64, :], a2w_v[0:64, :, 0])
            nc.vector.tensor_copy(a3[64:128, :], a2w_v[64:128, :, 1])
            # stacked-diagonal attention matrices: diag[:, 64k:64k+64]
            diag = mid.tile([128, 32 * 64], F16, tag="diag")
            for k in range(32):
                nc.vector.tensor_scalar_mul(
                    diag[:, 64 * k:64 * (k + 1)], I2_sb[:], a3[:, k:k + 1])

            # ---------- phase E: ct = Ahat^T @ encS -> [BC, H] ----------
            ct_sb = mid.tile([BC, H], F16, tag="ct")
            for n2 in range(2):
                ps = psS.tile([BC, 512], F32, tag="ps64", name="ctps")
                for k in range(32):
                    es = esp.tile([128, 512], F16, tag="encS")
                    nc.sync.dma_start(
                        es[:], d_encS[128 * k:128 * (k + 1),
                                      512 * n2:512 * (n2 + 1)])
                    nc.tensor.matmul(ps[:], diag[:, 64 * k:64 * (k + 1)],
                                     es[:], start=(k == 0), stop=(k == 31))
                nc.vector.tensor_copy(ct_sb[:, 512 * n2:512 * (n2 + 1)], ps[:])

            # ---------- phase F: ct^T into xT blocks 2..9 ----------
            for j in range(8):
                pt = psT.tile([128, BC], F16, tag="pT", name="trps")
                nc.tensor.transpose(pt[:], ct_sb[:, 128 * j:128 * (j + 1)],
                                    I128_sb[0:64, 0:64])
                nc.vector.tensor_copy(xT_sb[:, BC * (2 + j):BC * (3 + j)],
                                      pt[:])

            # ---------- phases G/H: two LSTM layers ----------
            def lstm_layer(xT, n_xk, wxd, whd, hT, bias_d, c_in,
                           d_hout, d_cout, hTout, lname):
                """xT: SBUF [128, n_xk*64] input^T blocks; whd/wxd DRAM weights;
                hT: SBUF [128, 8*64] prev-h^T blocks; returns nothing."""
                gates = mid.tile([BC, G4], F32, tag="gates", name="gates")
                for n in range(8):
                    ps = psS.tile([BC, 512], F32, tag="ps64", name="gps")
                    for k in range(n_xk):
                        wt = wpool.tile([128, 512], F16, tag="wtile")
                        nc.gpsimd.dma_start(
                            wt[:], wxd[128 * k:128 * (k + 1),
                                       512 * n:512 * (n + 1)])
                        nc.tensor.matmul(ps[:], xT[:, 64 * k:64 * (k + 1)],
                                         wt[:], start=(k == 0), stop=False)
                    for k in range(8):
                        wt = wpool.tile([128, 512], F16, tag="wtile")
                        nc.gpsimd.dma_start(
                            wt[:], whd[128 * k:128 * (k + 1),
                                       512 * n:512 * (n + 1)])
                        nc.tensor.matmul(ps[:], hT[:, 64 * k:64 * (k + 1)],
                                         wt[:], start=False, stop=False)
                    bt = wpool.tile([1, 512], F16, tag="wtile", name="bt")
                    nc.sync.dma_start(bt[:], bias_d[:, 512 * n:512 * (n + 1)])
                    nc.tensor.matmul(ps[:], ones1_sb[:], bt[:],
                                     start=False, stop=True)
                    func = AF.Tanh if n in (4, 5) else AF.Sigmoid
                    nc.scalar.activation(gates[:, 512 * n:512 * (n + 1)],
                                         ps[:], func)
                # c2 = sig_f*c + sig_i*tanh_g ; h2 = sig_o*tanh(c2)
                tmp = mid.tile([BC, H], F32, tag="lstm_tmp", name="tmp")
                nc.vector.tensor_tensor(tmp[:], gates[:, 0:H],
                                        gates[:, 2 * H:3 * H], ALU.mult)
                c2 = mid.tile([BC, H], F32, tag="c2t", name="c2")
                nc.vector.tensor_tensor(c2[:], gates[:, H:2 * H], c_in[:],
                                        ALU.mult)
                nc.vector.tensor_tensor(c2[:], c2[:], tmp[:], ALU.add)
                nc.sync.dma_start(d_cout[:], c2[:])
                tc2 = mid.tile([BC, H], F32, tag="lstm_tmp", name="tc2")
                nc.scalar.activation(tc2[:], c2[:], AF.Tanh)
                h2 = mid.tile([BC, H], F32, tag="h2t", name="h2")
                nc.vector.tensor_tensor(h2[:], gates[:, 3 * H:4 * H], tc2[:],
                                        ALU.mult)
                nc.sync.dma_start(d_hout[:], h2[:])
                h2f = mid.tile([BC, H], F16, tag="lstm_h2f", name="h2f")
                nc.vector.tensor_copy(h2f[:], h2[:])
                for j in range(8):
                    pt = psT.tile([128, BC], F16, tag="pT", name="trps")
                    nc.tensor.transpose(pt[:], h2f[:, 128 * j:128 * (j + 1)],
                                        I128_sb[0:64, 0:64])
                    nc.vector.tensor_copy(hTout[:, BC * j:BC * (j + 1)], pt[:])

            h0T_sb = mid.tile([128, 8 * BC], F16, tag="h0T")
            lstm_layer(xT_sb, 10, d_wi0, d_wh0, hT0_sb, d_b0, c0_sb,
                       d_h0n, d_c0n, h0T_sb, "l0")
            h1T_sb = mid.tile([128, 8 * BC], F16, tag="h1T")
            lstm_layer(h0T_sb, 8, d_wi1, d_wh1, hT1_sb, d_b1, c1_sb,
                       d_h1n, d_c1n, h1T_sb, "l1")

            # ---------- phase I: logits ----------
            pl = psS.tile([BC, V], F32, tag="ps64", name="lps")
            for k in range(8):
                nc.tensor.matmul(pl[:], h1T_sb[:, 64 * k:64 * (k + 1)],
                                 owT_sb[k][:], start=(k == 0), stop=False)
            nc.tensor.matmul(pl[:], ones1_sb[:], ob_sb[:],
                             start=False, stop=True)
            lo = mid.tile([BC, V], F32, tag="lo")
            nc.vector.tensor_copy(lo[:], pl[:])
            nc.sync.dma_start(d_logits[:], lo[:])

    nc.compile()
    return nc


def _prep_inputs(input_ids, hidden, cell, encoder_outputs, emb, U, W, Vw,
                 Wih0, Whh0, bih0, bhh0, Wih1, Whh1, bih1, bhh1,
                 out_w, out_b):
    f16 = np.float16
    # shared across cores
    U16 = np.ascontiguousarray(U.astype(f16))
    W16 = np.ascontiguousarray(W.astype(f16))
    VwR = np.ascontiguousarray(Vw.reshape(8, 128).T.astype(f16))  # [128,8]
    emb16 = np.ascontiguousarray(emb.astype(f16))
    oh64 = np.zeros((BC, TOK), f16)
    for b in range(BC):
        oh64[b, 64 * b:64 * (b + 1)] = 1.0
    I2 = np.zeros((128, 64), np.float32)
    I2[np.arange(128), np.arange(128) % 64] = 1.0
    I128 = np.eye(128, dtype=f16)
    ones1 = np.ones((1, BC), f16)
    Wih0T = np.ascontiguousarray(Wih0.T.astype(f16))
    Whh0T = np.ascontiguousarray(Whh0.T.astype(f16))
    Wih1T = np.ascontiguousarray(Wih1.T.astype(f16))
    Whh1T = np.ascontiguousarray(Whh1.T.astype(f16))
    b0 = np.ascontiguousarray((bih0 + bhh0)[None, :].astype(f16))
    b1 = np.ascontiguousarray((bih1 + bhh1)[None, :].astype(f16))
    owT = np.ascontiguousarray(out_w.T.astype(f16))
    ob = np.ascontiguousarray(out_b[None, :].astype(f16))

    def blocked_T(x):  # [BC,H] -> [128, 8*BC] (k-blocks of columns)
        t = np.ascontiguousarray(x.T)          # [H, BC]
        return np.ascontiguousarray(
            t.reshape(8, 128, BC).transpose(1, 0, 2).reshape(128, 8 * BC)
        ).astype(f16)

    ids = np.asarray(input_ids).reshape(B)
    in_maps = []
    for c in range(NCORES):
        bs = slice(BC * c, BC * (c + 1))
        enc_c = encoder_outputs[bs]                      # [BC, S, H]
        encT = np.ascontiguousarray(
            enc_c.reshape(TOK, H).T.astype(f16))         # [H, TOK] b-major
        encS = np.ascontiguousarray(
            enc_c.transpose(1, 0, 2).reshape(TOK, H).astype(f16))  # s-major
        ohT = np.zeros((V, BC), f16)
        ohT[ids[bs].astype(np.int64), np.arange(BC)] = 1.0
        in_maps.append({
            "encT": encT, "encS": encS, "Umat": U16, "Wmat": W16,
            "VwR": VwR,
            "hT0": blocked_T(hidden[0][bs]),
            "hT1": blocked_T(hidden[1][bs]),
            "onehotT": ohT, "embW": emb16, "oh64": oh64, "I2": I2,
            "I128": I128, "ones1": ones1,
            "Wih0T": Wih0T, "Whh0T": Whh0T, "Wih1T": Wih1T, "Whh1T": Whh1T,
            "bias0": b0, "bias1": b1,
            "cell0": np.ascontiguousarray(cell[0][bs], dtype=np.float32),
            "cell1": np.ascontiguousarray(cell[1][bs], dtype=np.float32),
            "outWT": owT, "outB": ob,
        })
    return in_maps


def kernel(input_ids, hidden, cell, encoder_outputs, emb, U, W, Vw,
           Wih0, Whh0, bih0, bhh0, Wih1, Whh1, bih1, bhh1,
           out_w, out_b, matrix=0, _trace=False):
    if _COMPILED[0] is None:
        _COMPILED[0] = _build()
    nc = _COMPILED[0]
    args = [np.asarray(a) for a in
            (input_ids, hidden, cell, encoder_outputs, emb, U, W, Vw,
             Wih0, Whh0, bih0, bhh0, Wih1, Whh1, bih1, bhh1, out_w, out_b)]
    in_maps = _prep_inputs(*args)
    res = run_bass_kernel_spmd(nc, in_maps, core_ids=list(range(NCORES)),
                               trace=_trace)
    outs = res.results
    logits = np.concatenate([outs[c]["logits"] for c in range(NCORES)], 0)
    h_new = np.stack([
        np.concatenate([outs[c]["h0n"] for c in range(NCORES)], 0),
        np.concatenate([outs[c]["h1n"] for c in range(NCORES)], 0)])
    c_new = np.stack([
        np.concatenate([outs[c]["c0n"] for c in range(NCORES)], 0),
        np.concatenate([outs[c]["c1n"] for c in range(NCORES)], 0)])
    out = logits[:, None, :].astype(np.float32)
    kernel._last_results = res
    if int(np.asarray(matrix)):
        raise NotImplementedError("matrix=1 path not needed (reference uses 0)")
    return (out, h_new.astype(np.float32), c_new.astype(np.float32))


# revision 12
# speedup vs baseline: 1.1950x; 1.1950x over previous
"""Trainium2 Bass kernel for one attention-LSTM decoder step.

dims: B=512, S=64, H=1024, E=256, V=128, L=2, sharded data-parallel over
batch across 8 NeuronCores (64 batches/core). All matmuls run in fp16 with
fp32 PSUM accumulation; elementwise/softmax math in fp32.
"""

import sys

if "/opt/trn_rl_repo" not in sys.path:
    sys.path.insert(0, "/opt/trn_rl_repo")

import numpy as np

import concourse.bacc as bacc
import concourse.mybir as mybir
import concourse.tile as tile
from concourse.bass_utils import run_bass_kernel_spmd

B, S, H, E, V = 512, 64, 1024, 256, 128
NCORES = 8
BC = B // NCORES          # 64 batches per core
TOK = BC * S              # 4096 tokens per core
F = E + H                 # 1280 LSTM input features
G4 = 4 * H                # 4096 gate rows
F16 = mybir.dt.float16
F32 = mybir.dt.float32
AF = mybir.ActivationFunctionType
ALU = mybir.AluOpType
AX = mybir.AxisListType

_COMPILED = [None]


def _build():
    nc = bacc.Bacc("TRN2", target_bir_lowering=False, debug=False,
                   num_devices=NCORES)

    # ---- DRAM I/O ----
    d_encT = nc.dram_tensor("encT", [H, TOK], F16, kind="ExternalInput")
    d_encS = nc.dram_tensor("encS", [TOK, H], F16, kind="ExternalInput")
    d_U = nc.dram_tensor("Umat", [H, H], F16, kind="ExternalInput")
    d_W = nc.dram_tensor("Wmat", [H, H], F16, kind="ExternalInput")
    d_Vw = nc.dram_tensor("VwR", [128, 8], F16, kind="ExternalInput")
    d_hT0 = nc.dram_tensor("hT0", [128, 8 * BC], F16, kind="ExternalInput")
    d_hT1 = nc.dram_tensor("hT1", [128, 8 * BC], F16, kind="ExternalInput")
    d_ohT = nc.dram_tensor("onehotT", [V, BC], F16, kind="ExternalInput")
    d_emb = nc.dram_tensor("embW", [V, E], F16, kind="ExternalInput")
    d_oh64 = nc.dram_tensor("oh64", [BC, TOK], F16, kind="ExternalInput")
    d_I2 = nc.dram_tensor("I2", [128, 64], F32, kind="ExternalInput")
    d_I128 = nc.dram_tensor("I128", [128, 128], F16, kind="ExternalInput")
    d_ones1 = nc.dram_tensor("ones1", [1, BC], F16, kind="ExternalInput")
    d_wi0 = nc.dram_tensor("Wih0T", [F, G4], F16, kind="ExternalInput")
    d_wh0 = nc.dram_tensor("Whh0T", [H, G4], F16, kind="ExternalInput")
    d_wi1 = nc.dram_tensor("Wih1T", [H, G4], F16, kind="ExternalInput")
    d_wh1 = nc.dram_tensor("Whh1T", [H, G4], F16, kind="ExternalInput")
    d_b0 = nc.dram_tensor("bias0", [1, G4], F16, kind="ExternalInput")
    d_b1 = nc.dram_tensor("bias1", [1, G4], F16, kind="ExternalInput")
    d_c0 = nc.dram_tensor("cell0", [BC, H], F32, kind="ExternalInput")
    d_c1 = nc.dram_tensor("cell1", [BC, H], F32, kind="ExternalInput")
    d_owT = nc.dram_tensor("outWT", [H, V], F16, kind="ExternalInput")
    d_ob = nc.dram_tensor("outB", [1, V], F16, kind="ExternalInput")

    d_logits = nc.dram_tensor("logits", [BC, V], F32, kind="ExternalOutput")
    d_h0n = nc.dram_tensor("h0n", [BC, H], F32, kind="ExternalOutput")
    d_h1n = nc.dram_tensor("h1n", [BC, H], F32, kind="ExternalOutput")
    d_c0n = nc.dram_tensor("c0n", [BC, H], F32, kind="ExternalOutput")
    d_c1n = nc.dram_tensor("c1n", [BC, H], F32, kind="ExternalOutput")

    with tile.TileContext(nc) as tc:
        with (
            tc.tile_pool(name="const", bufs=1) as cpool,
            tc.tile_pool(name="stream", bufs=3) as stp,
            tc.tile_pool(name="tanh", bufs=2) as thp,
            tc.tile_pool(name="wls", bufs=2) as wpool,
            tc.tile_pool(name="mid", bufs=1) as mid,
            tc.tile_pool(name="psA", bufs=2, space="PSUM") as psA,
            tc.tile_pool(name="psE", bufs=2, space="PSUM") as psE,
            tc.tile_pool(name="psS", bufs=2, space="PSUM") as psS,
            tc.tile_pool(name="psT", bufs=2, space="PSUM") as psT,
        ):
            # ---------- resident constants ----------
            U_sb = [cpool.tile([128, H], F16, tag=f"U{k}", name=f"U{k}") for k in range(8)]
            for k in range(8):
                nc.sync.dma_start(U_sb[k][:], d_U[128 * k:128 * (k + 1), :])
            vw_sb = cpool.tile([128, 8], F16, tag="vw")
            nc.sync.dma_start(vw_sb[:], d_Vw[:])
            hT0_sb = cpool.tile([128, 8 * BC], F16, tag="hT0")
            nc.sync.dma_start(hT0_sb[:], d_hT0[:])
            hT1_sb = cpool.tile([128, 8 * BC], F16, tag="hT1")
            nc.sync.dma_start(hT1_sb[:], d_hT1[:])
            ohT_sb = cpool.tile([V, BC], F16, tag="ohT")
            nc.sync.dma_start(ohT_sb[:], d_ohT[:])
            emb_sb = cpool.tile([V, E], F16, tag="emb")
            nc.sync.dma_start(emb_sb[:], d_emb[:])
            I2_sb = cpool.tile([128, 64], F32, tag="I2")
            nc.sync.dma_start(I2_sb[:], d_I2[:])
            I128_sb = cpool.tile([128, 128], F16, tag="I128")
            nc.sync.dma_start(I128_sb[:], d_I128[:])
            ones1_sb = cpool.tile([1, BC], F16, tag="ones1")
            nc.sync.dma_start(ones1_sb[:], d_ones1[:])
            c0_sb = cpool.tile([BC, H], F32, tag="c0")
            nc.sync.dma_start(c0_sb[:], d_c0[:])
            c1_sb = cpool.tile([BC, H], F32, tag="c1")
            nc.sync.dma_start(c1_sb[:], d_c1[:])
            owT_sb = [cpool.tile([128, V], F16, tag=f"ow{k}", name=f"ow{k}") for k in range(8)]
            for k in range(8):
                nc.sync.dma_start(owT_sb[k][:], d_owT[128 * k:128 * (k + 1), :])
            ob_sb = cpool.tile([1, V], F16, tag="ob")
            nc.sync.dma_start(ob_sb[:], d_ob[:])

            # ---------- phase A: t2 = h_top @ W  -> [BC, H] f16 ----------
            t2_sb = mid.tile([BC, H], F16, tag="t2")
            for n2 in range(2):
                ps = psS.tile([BC, 512], F32, tag="ps64", name="t2ps")
                wsl = stp.tile([128, 4096], F16, tag="big", name="wsl")
                nc.sync.dma_start(
                    wsl[:].rearrange("p (k c) -> p k c", c=512),
                    d_W.rearrange("(k p) h -> k p h", p=128)
                       [:, :, 512 * n2:512 * (n2 + 1)]
                       .transpose([1, 0, 2]))
                for k in range(8):
                    nc.tensor.matmul(
                        ps[:], hT1_sb[:, 64 * k:64 * (k + 1)],
                        wsl[:, 512 * k:512 * (k + 1)],
                        start=(k == 0), stop=(k == 7))
                nc.vector.tensor_copy(t2_sb[:, 512 * n2:512 * (n2 + 1)], ps[:])

            # ---------- phase B: embedded^T -> xT blocks 0..1 ----------
            xT_sb = mid.tile([128, 10 * BC], F16, tag="xT")
            for et in range(2):
                ps = psT.tile([128, BC], F32, tag="pT", name="embps")
                nc.tensor.matmul(ps[:], emb_sb[:, 128 * et:128 * (et + 1)],
                                 ohT_sb[:], start=True, stop=True)
                nc.vector.tensor_copy(xT_sb[:, BC * et:BC * (et + 1)], ps[:])

            # ---------- phase C: t1 + t2 -> tanh -> scores ----------
            e_row = mid.tile([1, TOK], F16, tag="e_row")
            for n in range(8):          # token tiles (512 tokens, 8 batches)
                et = stp.tile([128, 4096], F16, tag="big", name="et")
                nc.sync.dma_start(
                    et[:].rearrange("p (k c) -> p k c", c=512),
                    d_encT.rearrange("(k p) t -> k p t", p=128)
                          [:, :, 512 * n:512 * (n + 1)]
                          .transpose([1, 0, 2]))
                oh64_t = stp.tile([BC, 512], F16, tag="oh", name="oh64_t")
                nc.sync.dma_start(oh64_t[:], d_oh64[:, 512 * n:512 * (n + 1)])
                pe = psE.tile([1, 512], F32, tag="eps")
                for m in range(8):      # output-H tiles
                    pt = psA.tile([128, 512], F32, tag="t1ps")
                    for k in range(8):
                        nc.tensor.matmul(
                            pt[:], U_sb[k][:, 128 * m:128 * (m + 1)],
                            et[:, 512 * k:512 * (k + 1)],
                            start=(k == 0), stop=False)
                    # inject t2 broadcast over s:  lhsT=[64b,128h'] rhs=[64b,512tok]
                    nc.tensor.matmul(
                        pt[:], t2_sb[:, 128 * m:128 * (m + 1)],
                        oh64_t[:], start=False, stop=True)
                    th = thp.tile([128, 512], F16, tag="tanh")
                    nc.scalar.activation(th[:], pt[:], AF.Tanh)
                    nc.tensor.matmul(pe[:], vw_sb[:, m:m + 1], th[:],
                                     start=(m == 0), stop=(m == 7))
                nc.vector.tensor_copy(e_row[:, 512 * n:512 * (n + 1)], pe[:])

            # ---------- phase D: softmax over s (rows b, duplicated halves) --
            a2 = mid.tile([128, S], F32, tag="a2")
            src = e_row[0:1, :].rearrange("p (b s) -> p b s", b=BC)
            nc.gpsimd.dma_start(a2[0:BC, :], src)
            nc.gpsimd.dma_start(a2[BC:128, :], src)
            mx = mid.tile([128, 1], F32, tag="mx")
            nc.vector.tensor_reduce(mx[:], a2[:], axis=AX.X, op=ALU.max)
            negmx = mid.tile([128, 1], F32, tag="negmx")
            nc.vector.tensor_scalar_mul(negmx[:], mx[:], -1.0)
            p2 = mid.tile([128, S], F32, tag="p2")
            nc.scalar.activation(p2[:], a2[:], AF.Exp, bias=negmx[:])
            sm = mid.tile([128, 1], F32, tag="sm")
            nc.vector.tensor_reduce(sm[:], p2[:], axis=AX.X, op=ALU.add)
            rinv = mid.tile([128, 1], F32, tag="rinv")
            nc.vector.reciprocal(rinv[:], sm[:])
            a2w = mid.tile([128, S], F32, tag="a2w")
            nc.vector.tensor_scalar_mul(a2w[:], p2[:], rinv[:])
            # a3[p,k] = a2w[p, 2k + (p>=64)]
            a3 = mid.tile([128, 32], F32, tag="a3")
            a2w_v = a2w[:].rearrange("p (k two) -> p k two", two=2)
            nc.vector.tensor_copy(a3[0:64, :], a2w_v[0:64, :, 0])
            nc.vector.tensor_copy(a3[64:128, :], a2w_v[64:128, :, 1])
            # stacked-diagonal attention matrices: diag[:, 64k:64k+64]
            diag = mid.tile([128, 32 * 64], F16, tag="diag")
            for k in range(32):
                nc.vector.tensor_scalar_mul(
                    diag[:, 64 * k:64 * (k + 1)], I2_sb[:], a3[:, k:k + 1])

            # ---------- phase E: ct = Ahat^T @ encS -> [BC, H] ----------
            ct_sb = mid.tile([BC, H], F16, tag="ct")
            for n2 in range(2):
                ps = psS.tile([BC, 512], F32, tag="ps64", name="ctps")
                for kc in range(4):
                    es = stp.tile([128, 4096], F16, tag="big", name="es")
                    nc.sync.dma_start(
                        es[:].rearrange("p (k c) -> p k c", c=512),
                        d_encS.rearrange("(kc k p) h -> kc k p h", kc=4, p=128)
                              [kc, :, :, 512 * n2:512 * (n2 + 1)]
                              .transpose([1, 0, 2]))
                    for kk in range(8):
                        k = 8 * kc + kk
                        nc.tensor.matmul(ps[:], diag[:, 64 * k:64 * (k + 1)],
                                         es[:, 512 * kk:512 * (kk + 1)],
                                         start=(k == 0), stop=(k == 31))
                nc.vector.tensor_copy(ct_sb[:, 512 * n2:512 * (n2 + 1)], ps[:])

            # ---------- phase F: ct^T into xT blocks 2..9 ----------
            for j in range(8):
                pt = psT.tile([128, BC], F16, tag="pT", name="trps")
                nc.tensor.transpose(pt[:], ct_sb[:, 128 * j:128 * (j + 1)],
                                    I128_sb[0:64, 0:64])
                nc.vector.tensor_copy(xT_sb[:, BC * (2 + j):BC * (3 + j)],
                                      pt[:])

            # ---------- phases G/H: two LSTM layers ----------
            def lstm_layer(xT, n_xk, wxd, whd, hT, bias_d, c_in,
                           d_hout, d_cout, hTout, lname):
                """xT: SBUF [128, n_xk*64] input^T blocks; whd/wxd DRAM weights;
                hT: SBUF [128, 8*64] prev-h^T blocks; returns nothing."""
                gates = mid.tile([BC, G4], F16, tag="gates", name="gates")
                bt = mid.tile([1, G4], F16, tag="btile", name="bt")
                nc.sync.dma_start(bt[:], bias_d[:])
                wx_v = wxd.rearrange("(k p) g -> k p g", p=128)
                wh_v = whd.rearrange("(k p) g -> k p g", p=128)
                for n in range(8):
                    # batched loads: all k-slices of this 512-gate column
                    wx = wpool.tile([128, n_xk * 512], F16, tag="wx", name="wx")
                    nc.scalar.dma_start(
                        wx[:].rearrange("p (k c) -> p k c", c=512),
                        wx_v[:, :, 512 * n:512 * (n + 1)].transpose([1, 0, 2]))
                    wh = wpool.tile([128, 8 * 512], F16, tag="wh", name="wh")
                    nc.scalar.dma_start(
                        wh[:].rearrange("p (k c) -> p k c", c=512),
                        wh_v[:, :, 512 * n:512 * (n + 1)].transpose([1, 0, 2]))
                    ps = psS.tile([BC, 512], F32, tag="ps64", name="gps")
                    for k in range(n_xk):
                        nc.tensor.matmul(ps[:], xT[:, 64 * k:64 * (k + 1)],
                                         wx[:, 512 * k:512 * (k + 1)],
                                         start=(k == 0), stop=False)
                    for k in range(8):
                        nc.tensor.matmul(ps[:], hT[:, 64 * k:64 * (k + 1)],
                                         wh[:, 512 * k:512 * (k + 1)],
                                         start=False, stop=False)
                    nc.tensor.matmul(ps[:], ones1_sb[:],
                                     bt[:, 512 * n:512 * (n + 1)],
                                     start=False, stop=True)
                    func = AF.Tanh if n in (4, 5) else AF.Sigmoid
                    nc.scalar.activation(gates[:, 512 * n:512 * (n + 1)],
                                         ps[:], func)
                # c2 = sig_f*c + sig_i*tanh_g ; h2 = sig_o*tanh(c2)
                tmp = mid.tile([BC, H], F32, tag="lstm_tmp", name="tmp")
                nc.vector.tensor_tensor(tmp[:], gates[:, 0:H],
                                        gates[:, 2 * H:3 * H], ALU.mult)
                c2 = mid.tile([BC, H], F32, tag="c2t", name="c2")
                nc.vector.tensor_tensor(c2[:], gates[:, H:2 * H], c_in[:],
                                        ALU.mult)
                nc.vector.tensor_tensor(c2[:], c2[:], tmp[:], ALU.add)
                nc.sync.dma_start(d_cout[:], c2[:])
                tc2 = mid.tile([BC, H], F32, tag="lstm_tmp", name="tc2")
                nc.scalar.activation(tc2[:], c2[:], AF.Tanh)
                h2 = mid.tile([BC, H], F32, tag="h2t", name="h2")
                nc.vector.tensor_tensor(h2[:], gates[:, 3 * H:4 * H], tc2[:],
                                        ALU.mult)
                nc.sync.dma_start(d_hout[:], h2[:])
                h2f = mid.tile([BC, H], F16, tag="lstm_h2f", name="h2f")
                nc.vector.tensor_copy(h2f[:], h2[:])
                for j in range(8):
                    pt = psT.tile([128, BC], F16, tag="pT", name="trps")
                    nc.tensor.transpose(pt[:], h2f[:, 128 * j:128 * (j + 1)],
                                        I128_sb[0:64, 0:64])
                    nc.vector.tensor_copy(hTout[:, BC * j:BC * (j + 1)], pt[:])

            h0T_sb = mid.tile([128, 8 * BC], F16, tag="h0T")
            lstm_layer(xT_sb, 10, d_wi0, d_wh0, hT0_sb, d_b0, c0_sb,
                       d_h0n, d_c0n, h0T_sb, "l0")
            h1T_sb = mid.tile([128, 8 * BC], F16, tag="h1T")
            lstm_layer(h0T_sb, 8, d_wi1, d_wh1, hT1_sb, d_b1, c1_sb,
                       d_h1n, d_c1n, h1T_sb, "l1")

            # ---------- phase I: logits ----------
            pl = psS.tile([BC, V], F32, tag="ps64", name="lps")
            for k in range(8):
                nc.tensor.matmul(pl[:], h1T_sb[:, 64 * k:64 * (k + 1)],
                                 owT_sb[k][:], start=(k == 0), stop=False)
            nc.tensor.matmul(pl[:], ones1_sb[:], ob_sb[:],
                             start=False, stop=True)
            lo = mid.tile([BC, V], F32, tag="lo")
            nc.vector.tensor_copy(lo[:], pl[:])
            nc.sync.dma_start(d_logits[:], lo[:])

    nc.compile()
    return nc


def _prep_inputs(input_ids, hidden, cell, encoder_outputs, emb, U, W, Vw,
                 Wih0, Whh0, bih0, bhh0, Wih1, Whh1, bih1, bhh1,
                 out_w, out_b):
    f16 = np.float16
    # shared across cores
    U16 = np.ascontiguousarray(U.astype(f16))
    W16 = np.ascontiguousarray(W.astype(f16))
    VwR = np.ascontiguousarray(Vw.reshape(8, 128).T.astype(f16))  # [128,8]
    emb16 = np.ascontiguousarray(emb.astype(f16))
    oh64 = np.zeros((BC, TOK), f16)
    for b in range(BC):
        oh64[b, 64 * b:64 * (b + 1)] = 1.0
    I2 = np.zeros((128, 64), np.float32)
    I2[np.arange(128), np.arange(128) % 64] = 1.0
    I128 = np.eye(128, dtype=f16)
    ones1 = np.ones((1, BC), f16)
    Wih0T = np.ascontiguousarray(Wih0.T.astype(f16))
    Whh0T = np.ascontiguousarray(Whh0.T.astype(f16))
    Wih1T = np.ascontiguousarray(Wih1.T.astype(f16))
    Whh1T = np.ascontiguousarray(Whh1.T.astype(f16))
    b0 = np.ascontiguousarray((bih0 + bhh0)[None, :].astype(f16))
    b1 = np.ascontiguousarray((bih1 + bhh1)[None, :].astype(f16))
    owT = np.ascontiguousarray(out_w.T.astype(f16))
    ob = np.ascontiguousarray(out_b[None, :].astype(f16))

    def blocked_T(x):  # [BC,H] -> [128, 8*BC] (k-blocks of columns)
        t = np.ascontiguousarray(x.T)          # [H, BC]
        return np.ascontiguousarray(
            t.reshape(8, 128, BC).transpose(1, 0, 2).reshape(128, 8 * BC)
        ).astype(f16)

    ids = np.asarray(input_ids).reshape(B)
    in_maps = []
    for c in range(NCORES):
        bs = slice(BC * c, BC * (c + 1))
        enc_c = encoder_outputs[bs]                      # [BC, S, H]
        encT = np.ascontiguousarray(
            enc_c.reshape(TOK, H).T.astype(f16))         # [H, TOK] b-major
        encS = np.ascontiguousarray(
            enc_c.transpose(1, 0, 2).reshape(TOK, H).astype(f16))  # s-major
        ohT = np.zeros((V, BC), f16)
        ohT[ids[bs].astype(np.int64), np.arange(BC)] = 1.0
        in_maps.append({
            "encT": encT, "encS": encS, "Umat": U16, "Wmat": W16,
            "VwR": VwR,
            "hT0": blocked_T(hidden[0][bs]),
            "hT1": blocked_T(hidden[1][bs]),
            "onehotT": ohT, "embW": emb16, "oh64": oh64, "I2": I2,
            "I128": I128, "ones1": ones1,
            "Wih0T": Wih0T, "Whh0T": Whh0T, "Wih1T": Wih1T, "Whh1T": Whh1T,
            "bias0": b0, "bias1": b1,
            "cell0": np.ascontiguousarray(cell[0][bs], dtype=np.float32),
            "cell1": np.ascontiguousarray(cell[1][bs], dtype=np.float32),
            "outWT": owT, "outB": ob,
        })
    return in_maps


def kernel(input_ids, hidden, cell, encoder_outputs, emb, U, W, Vw,
           Wih0, Whh0, bih0, bhh0, Wih1, Whh1, bih1, bhh1,
           out_w, out_b, matrix=0, _trace=False):
    if _COMPILED[0] is None:
        _COMPILED[0] = _build()
    nc = _COMPILED[0]
    args = [np.asarray(a) for a in
            (input_ids, hidden, cell, encoder_outputs, emb, U, W, Vw,
             Wih0, Whh0, bih0, bhh0, Wih1, Whh1, bih1, bhh1, out_w, out_b)]
    in_maps = _prep_inputs(*args)
    res = run_bass_kernel_spmd(nc, in_maps, core_ids=list(range(NCORES)),
                               trace=_trace)
    outs = res.results
    logits = np.concatenate([outs[c]["logits"] for c in range(NCORES)], 0)
    h_new = np.stack([
        np.concatenate([outs[c]["h0n"] for c in range(NCORES)], 0),
        np.concatenate([outs[c]["h1n"] for c in range(NCORES)], 0)])
    c_new = np.stack([
        np.concatenate([outs[c]["c0n"] for c in range(NCORES)], 0),
        np.concatenate([outs[c]["c1n"] for c in range(NCORES)], 0)])
    out = logits[:, None, :].astype(np.float32)
    kernel._last_results = res
    if int(np.asarray(matrix)):
        raise NotImplementedError("matrix=1 path not needed (reference uses 0)")
    return (out, h_new.astype(np.float32), c_new.astype(np.float32))


# revision 13
# speedup vs baseline: 1.3606x; 1.1386x over previous
"""Trainium2 Bass kernel for one attention-LSTM decoder step.

dims: B=512, S=64, H=1024, E=256, V=128, L=2, sharded data-parallel over
batch across 8 NeuronCores (64 batches/core). All matmuls run in fp16 with
fp32 PSUM accumulation; elementwise/softmax math in fp32.
"""

import sys

if "/opt/trn_rl_repo" not in sys.path:
    sys.path.insert(0, "/opt/trn_rl_repo")

import numpy as np

import concourse.bacc as bacc
import concourse.mybir as mybir
import concourse.tile as tile
from concourse.bass_utils import run_bass_kernel_spmd

B, S, H, E, V = 512, 64, 1024, 256, 128
NCORES = 8
BC = B // NCORES          # 64 batches per core
TOK = BC * S              # 4096 tokens per core
F = E + H                 # 1280 LSTM input features
G4 = 4 * H                # 4096 gate rows
F16 = mybir.dt.float16
F32 = mybir.dt.float32
AF = mybir.ActivationFunctionType
ALU = mybir.AluOpType
AX = mybir.AxisListType

_COMPILED = [None]


def _build():
    nc = bacc.Bacc("TRN2", target_bir_lowering=False, debug=False,
                   num_devices=NCORES)

    # ---- DRAM I/O ----
    d_encT = nc.dram_tensor("encT", [H, TOK], F16, kind="ExternalInput")
    d_encS = nc.dram_tensor("encS", [TOK, H], F16, kind="ExternalInput")
    d_U = nc.dram_tensor("Umat", [H, H], F16, kind="ExternalInput")
    d_W = nc.dram_tensor("Wmat", [H, H], F16, kind="ExternalInput")
    d_Vw = nc.dram_tensor("VwR", [128, 8], F16, kind="ExternalInput")
    d_hT0 = nc.dram_tensor("hT0", [128, 8 * BC], F16, kind="ExternalInput")
    d_hT1 = nc.dram_tensor("hT1", [128, 8 * BC], F16, kind="ExternalInput")
    d_ohT = nc.dram_tensor("onehotT", [V, BC], F16, kind="ExternalInput")
    d_emb = nc.dram_tensor("embW", [V, E], F16, kind="ExternalInput")
    d_oh64 = nc.dram_tensor("oh64", [BC, TOK], F16, kind="ExternalInput")
    d_I2 = nc.dram_tensor("I2", [128, 64], F32, kind="ExternalInput")
    d_I128 = nc.dram_tensor("I128", [128, 128], F16, kind="ExternalInput")
    d_ones1 = nc.dram_tensor("ones1", [1, BC], F16, kind="ExternalInput")
    d_wi0 = nc.dram_tensor("Wih0T", [F, G4], F16, kind="ExternalInput")
    d_wh0 = nc.dram_tensor("Whh0T", [H, G4], F16, kind="ExternalInput")
    d_wi1 = nc.dram_tensor("Wih1T", [H, G4], F16, kind="ExternalInput")
    d_wh1 = nc.dram_tensor("Whh1T", [H, G4], F16, kind="ExternalInput")
    d_b0 = nc.dram_tensor("bias0", [1, G4], F16, kind="ExternalInput")
    d_b1 = nc.dram_tensor("bias1", [1, G4], F16, kind="ExternalInput")
    d_c0 = nc.dram_tensor("cell0", [BC, H], F32, kind="ExternalInput")
    d_c1 = nc.dram_tensor("cell1", [BC, H], F32, kind="ExternalInput")
    d_owT = nc.dram_tensor("outWT", [H, V], F16, kind="ExternalInput")
    d_ob = nc.dram_tensor("outB", [1, V], F16, kind="ExternalInput")

    d_logits = nc.dram_tensor("logits", [BC, V], F32, kind="ExternalOutput")
    d_h0n = nc.dram_tensor("h0n", [BC, H], F32, kind="ExternalOutput")
    d_h1n = nc.dram_tensor("h1n", [BC, H], F32, kind="ExternalOutput")
    d_c0n = nc.dram_tensor("c0n", [BC, H], F32, kind="ExternalOutput")
    d_c1n = nc.dram_tensor("c1n", [BC, H], F32, kind="ExternalOutput")

    with tile.TileContext(nc) as tc:
        with (
            tc.tile_pool(name="const", bufs=1) as cpool,
            tc.tile_pool(name="stream", bufs=2) as stp,
            tc.tile_pool(name="es2", bufs=4) as esp,
            tc.tile_pool(name="tanh", bufs=2) as thp,
            tc.tile_pool(name="wls", bufs=2) as wpool,
            tc.tile_pool(name="mid", bufs=1) as mid,
            tc.tile_pool(name="psA", bufs=2, space="PSUM") as psA,
            tc.tile_pool(name="psE", bufs=2, space="PSUM") as psE,
            tc.tile_pool(name="psS", bufs=2, space="PSUM") as psS,
            tc.tile_pool(name="psT", bufs=2, space="PSUM") as psT,
        ):
            # ---------- resident constants ----------
            vw_sb = cpool.tile([128, 8], F16, tag="vw")
            nc.sync.dma_start(vw_sb[:], d_Vw[:])
            hT0_sb = cpool.tile([128, 8 * BC], F16, tag="hT0")
            nc.sync.dma_start(hT0_sb[:], d_hT0[:])
            hT1_sb = cpool.tile([128, 8 * BC], F16, tag="hT1")
            nc.sync.dma_start(hT1_sb[:], d_hT1[:])
            ohT_sb = cpool.tile([V, BC], F16, tag="ohT")
            nc.sync.dma_start(ohT_sb[:], d_ohT[:])
            emb_sb = cpool.tile([V, E], F16, tag="emb")
            nc.sync.dma_start(emb_sb[:], d_emb[:])
            I2_sb = cpool.tile([128, 64], F32, tag="I2")
            nc.sync.dma_start(I2_sb[:], d_I2[:])
            ones1_sb = cpool.tile([1, BC], F16, tag="ones1")
            nc.sync.dma_start(ones1_sb[:], d_ones1[:])

            # ---------- phase A: t2 = h_top @ W  -> [BC, H] f16 ----------
            t2_sb = mid.tile([BC, H], F16, tag="t2")
            for n2 in range(2):
                ps = psS.tile([BC, 512], F32, tag="ps64", name="t2ps")
                wsl = stp.tile([128, 4096], F16, tag="big", name="wsl")
                nc.sync.dma_start(
                    wsl[:].rearrange("p (k c) -> p k c", c=512),
                    d_W.rearrange("(k p) h -> k p h", p=128)
                       [:, :, 512 * n2:512 * (n2 + 1)]
                       .transpose([1, 0, 2]))
                for k in range(8):
                    nc.tensor.matmul(
                        ps[:], hT1_sb[:, 64 * k:64 * (k + 1)],
                        wsl[:, 512 * k:512 * (k + 1)],
                        start=(k == 0), stop=(k == 7))
                nc.vector.tensor_copy(t2_sb[:, 512 * n2:512 * (n2 + 1)], ps[:])

            U_sb = [cpool.tile([128, H], F16, tag=f"U{k}", name=f"U{k}") for k in range(8)]
            for k in range(8):
                nc.sync.dma_start(U_sb[k][:], d_U[128 * k:128 * (k + 1), :])
            # ---------- phase B: embedded^T -> xT blocks 0..1 ----------
            xT_sb = mid.tile([128, 10 * BC], F16, tag="xT")
            for et in range(2):
                ps = psT.tile([128, BC], F32, tag="pT", name="embps")
                nc.tensor.matmul(ps[:], emb_sb[:, 128 * et:128 * (et + 1)],
                                 ohT_sb[:], start=True, stop=True)
                nc.vector.tensor_copy(xT_sb[:, BC * et:BC * (et + 1)], ps[:])

            I128_sb = cpool.tile([128, 128], F16, tag="I128")
            nc.sync.dma_start(I128_sb[:], d_I128[:])
            c0_sb = cpool.tile([BC, H], F32, tag="c0")
            nc.sync.dma_start(c0_sb[:], d_c0[:])
            c1_sb = cpool.tile([BC, H], F32, tag="c1")
            nc.sync.dma_start(c1_sb[:], d_c1[:])
            owT_sb = [cpool.tile([128, V], F16, tag=f"ow{k}", name=f"ow{k}") for k in range(8)]
            for k in range(8):
                nc.sync.dma_start(owT_sb[k][:], d_owT[128 * k:128 * (k + 1), :])
            ob_sb = cpool.tile([1, V], F16, tag="ob")
            nc.sync.dma_start(ob_sb[:], d_ob[:])
            # ---------- phase C: t1 + t2 -> tanh -> scores ----------
            e_row = mid.tile([1, TOK], F16, tag="e_row")
            for n in range(8):          # token tiles (512 tokens, 8 batches)
                et = stp.tile([128, 4096], F16, tag="big", name="et")
                nc.sync.dma_start(
                    et[:].rearrange("p (k c) -> p k c", c=512),
                    d_encT.rearrange("(k p) t -> k p t", p=128)
                          [:, :, 512 * n:512 * (n + 1)]
                          .transpose([1, 0, 2]))
                oh64_t = stp.tile([BC, 512], F16, tag="oh", name="oh64_t")
                nc.sync.dma_start(oh64_t[:], d_oh64[:, 512 * n:512 * (n + 1)])
                pe = psE.tile([1, 512], F32, tag="eps")
                for m in range(8):      # output-H tiles
                    pt = psA.tile([128, 512], F32, tag="t1ps")
                    for k in range(8):
                        nc.tensor.matmul(
                            pt[:], U_sb[k][:, 128 * m:128 * (m + 1)],
                            et[:, 512 * k:512 * (k + 1)],
                            start=(k == 0), stop=False)
                    # inject t2 broadcast over s:  lhsT=[64b,128h'] rhs=[64b,512tok]
                    nc.tensor.matmul(
                        pt[:], t2_sb[:, 128 * m:128 * (m + 1)],
                        oh64_t[:], start=False, stop=True)
                    th = thp.tile([128, 512], F16, tag="tanh")
                    nc.scalar.activation(th[:], pt[:], AF.Tanh)
                    nc.tensor.matmul(pe[:], vw_sb[:, m:m + 1], th[:],
                                     start=(m == 0), stop=(m == 7))
                nc.vector.tensor_copy(e_row[:, 512 * n:512 * (n + 1)], pe[:])

            # ---------- phase D: softmax over s (rows b, duplicated halves) --
            a2 = mid.tile([128, S], F32, tag="a2")
            src = e_row[0:1, :].rearrange("p (b s) -> p b s", b=BC)
            nc.gpsimd.dma_start(a2[0:BC, :], src)
            nc.gpsimd.dma_start(a2[BC:128, :], src)
            mx = mid.tile([128, 1], F32, tag="mx")
            nc.vector.tensor_reduce(mx[:], a2[:], axis=AX.X, op=ALU.max)
            negmx = mid.tile([128, 1], F32, tag="negmx")
            nc.vector.tensor_scalar_mul(negmx[:], mx[:], -1.0)
            p2 = mid.tile([128, S], F32, tag="p2")
            nc.scalar.activation(p2[:], a2[:], AF.Exp, bias=negmx[:])
            sm = mid.tile([128, 1], F32, tag="sm")
            nc.vector.tensor_reduce(sm[:], p2[:], axis=AX.X, op=ALU.add)
            rinv = mid.tile([128, 1], F32, tag="rinv")
            nc.vector.reciprocal(rinv[:], sm[:])
            a2w = mid.tile([128, S], F32, tag="a2w")
            nc.vector.tensor_scalar_mul(a2w[:], p2[:], rinv[:])
            # a3[p,k] = a2w[p, 2k + (p>=64)]
            a3 = mid.tile([128, 32], F32, tag="a3")
            a2w_v = a2w[:].rearrange("p (k two) -> p k two", two=2)
            nc.vector.tensor_copy(a3[0:64, :], a2w_v[0:64, :, 0])
            nc.vector.tensor_copy(a3[64:128, :], a2w_v[64:128, :, 1])
            # stacked-diagonal attention matrices: diag[:, 64k:64k+64]
            diag = mid.tile([128, 32 * 64], F16, tag="diag")
            i2b = I2_sb[:].unsqueeze(1).broadcast_to([128, 32, 64])
            a3b = a3[:].unsqueeze(2).broadcast_to([128, 32, 64])
            nc.vector.tensor_tensor(
                diag[:].rearrange("p (k c) -> p k c", c=64), i2b, a3b, ALU.mult)

            # ---------- phase E: ct = Ahat^T @ encS -> [BC, H] ----------
            ct_sb = mid.tile([BC, H], F16, tag="ct")
            for n2 in range(2):
                ps = psS.tile([BC, 512], F32, tag="ps64", name="ctps")
                for kc in range(8):
                    es = esp.tile([128, 2048], F16, tag="es", name="es")
                    nc.sync.dma_start(
                        es[:].rearrange("p (k c) -> p k c", c=512),
                        d_encS.rearrange("(kc k p) h -> kc k p h", kc=8, p=128)
                              [kc, :, :, 512 * n2:512 * (n2 + 1)]
                              .transpose([1, 0, 2]))
                    for kk in range(4):
                        k = 4 * kc + kk
                        nc.tensor.matmul(ps[:], diag[:, 64 * k:64 * (k + 1)],
                                         es[:, 512 * kk:512 * (kk + 1)],
                                         start=(k == 0), stop=(k == 31))
                nc.vector.tensor_copy(ct_sb[:, 512 * n2:512 * (n2 + 1)], ps[:])

            # ---------- phase F: ct^T into xT blocks 2..9 ----------
            for j in range(8):
                pt = psT.tile([128, BC], F16, tag="pT", name="trps")
                nc.tensor.transpose(pt[:], ct_sb[:, 128 * j:128 * (j + 1)],
                                    I128_sb[0:64, 0:64])
                nc.vector.tensor_copy(xT_sb[:, BC * (2 + j):BC * (3 + j)],
                                      pt[:])

            # ---------- phases G/H: two LSTM layers ----------
            def lstm_layer(xT, n_xk, wxd, whd, hT, bias_d, c_in,
                           d_hout, d_cout, hTout, lname):
                """xT: SBUF [128, n_xk*64] input^T blocks; whd/wxd DRAM weights;
                hT: SBUF [128, 8*64] prev-h^T blocks; returns nothing."""
                gates = mid.tile([BC, G4], F16, tag="gates", name="gates")
                bt = mid.tile([1, G4], F16, tag="btile", name="bt")
                nc.sync.dma_start(bt[:], bias_d[:])
                wx_v = wxd.rearrange("(k p) g -> k p g", p=128)
                wh_v = whd.rearrange("(k p) g -> k p g", p=128)
                for n in range(8):
                    # batched loads: all k-slices of this 512-gate column
                    wx = wpool.tile([128, n_xk * 512], F16, tag="wx", name="wx")
                    nc.scalar.dma_start(
                        wx[:].rearrange("p (k c) -> p k c", c=512),
                        wx_v[:, :, 512 * n:512 * (n + 1)].transpose([1, 0, 2]))
                    wh = wpool.tile([128, 8 * 512], F16, tag="wh", name="wh")
                    nc.sync.dma_start(
                        wh[:].rearrange("p (k c) -> p k c", c=512),
                        wh_v[:, :, 512 * n:512 * (n + 1)].transpose([1, 0, 2]))
                    ps = psS.tile([BC, 512], F32, tag="ps64", name="gps")
                    for k in range(n_xk):
                        nc.tensor.matmul(ps[:], xT[:, 64 * k:64 * (k + 1)],
                                         wx[:, 512 * k:512 * (k + 1)],
                                         start=(k == 0), stop=False)
                    for k in range(8):
                        nc.tensor.matmul(ps[:], hT[:, 64 * k:64 * (k + 1)],
                                         wh[:, 512 * k:512 * (k + 1)],
                                         start=False, stop=False)
                    nc.tensor.matmul(ps[:], ones1_sb[:],
                                     bt[:, 512 * n:512 * (n + 1)],
                                     start=False, stop=True)
                    func = AF.Tanh if n in (4, 5) else AF.Sigmoid
                    nc.scalar.activation(gates[:, 512 * n:512 * (n + 1)],
                                         ps[:], func)
                # c2 = sig_f*c + sig_i*tanh_g ; h2 = sig_o*tanh(c2)
                tmp = mid.tile([BC, H], F32, tag="lstm_tmp", name="tmp")
                nc.vector.tensor_tensor(tmp[:], gates[:, 0:H],
                                        gates[:, 2 * H:3 * H], ALU.mult)
                c2 = mid.tile([BC, H], F32, tag="c2t", name="c2")
                nc.vector.tensor_tensor(c2[:], gates[:, H:2 * H], c_in[:],
                                        ALU.mult)
                nc.vector.tensor_tensor(c2[:], c2[:], tmp[:], ALU.add)
                nc.sync.dma_start(d_cout[:], c2[:])
                tc2 = mid.tile([BC, H], F32, tag="lstm_tmp", name="tc2")
                nc.scalar.activation(tc2[:], c2[:], AF.Tanh)
                h2 = mid.tile([BC, H], F32, tag="h2t", name="h2")
                nc.vector.tensor_tensor(h2[:], gates[:, 3 * H:4 * H], tc2[:],
                                        ALU.mult)
                nc.sync.dma_start(d_hout[:], h2[:])
                h2f = mid.tile([BC, H], F16, tag="lstm_h2f", name="h2f")
                nc.vector.tensor_copy(h2f[:], h2[:])
                for j in range(8):
                    pt = psT.tile([128, BC], F16, tag="pT", name="trps")
                    nc.tensor.transpose(pt[:], h2f[:, 128 * j:128 * (j + 1)],
                                        I128_sb[0:64, 0:64])
                    nc.vector.tensor_copy(hTout[:, BC * j:BC * (j + 1)], pt[:])

            h0T_sb = mid.tile([128, 8 * BC], F16, tag="h0T")
            lstm_layer(xT_sb, 10, d_wi0, d_wh0, hT0_sb, d_b0, c0_sb,
                       d_h0n, d_c0n, h0T_sb, "l0")
            h1T_sb = mid.tile([128, 8 * BC], F16, tag="h1T")
            lstm_layer(h0T_sb, 8, d_wi1, d_wh1, hT1_sb, d_b1, c1_sb,
                       d_h1n, d_c1n, h1T_sb, "l1")

            # ---------- phase I: logits ----------
            pl = psS.tile([BC, V], F32, tag="ps64", name="lps")
            for k in range(8):
                nc.tensor.matmul(pl[:], h1T_sb[:, 64 * k:64 * (k + 1)],
                                 owT_sb[k][:], start=(k == 0), stop=False)
            nc.tensor.matmul(pl[:], ones1_sb[:], ob_sb[:],
                             start=False, stop=True)
            lo = mid.tile([BC, V], F32, tag="lo")
            nc.vector.tensor_copy(lo[:], pl[:])
            nc.sync.dma_start(d_logits[:], lo[:])

    nc.compile()
    return nc


def _prep_inputs(input_ids, hidden, cell, encoder_outputs, emb, U, W, Vw,
                 Wih0, Whh0, bih0, bhh0, Wih1, Whh1, bih1, bhh1,
                 out_w, out_b):
    f16 = np.float16
    # shared across cores
    U16 = np.ascontiguousarray(U.astype(f16))
    W16 = np.ascontiguousarray(W.astype(f16))
    VwR = np.ascontiguousarray(Vw.reshape(8, 128).T.astype(f16))  # [128,8]
    emb16 = np.ascontiguousarray(emb.astype(f16))
    oh64 = np.zeros((BC, TOK), f16)
    for b in range(BC):
        oh64[b, 64 * b:64 * (b + 1)] = 1.0
    I2 = np.zeros((128, 64), np.float32)
    I2[np.arange(128), np.arange(128) % 64] = 1.0
    I128 = np.eye(128, dtype=f16)
    ones1 = np.ones((1, BC), f16)
    Wih0T = np.ascontiguousarray(Wih0.T.astype(f16))
    Whh0T = np.ascontiguousarray(Whh0.T.astype(f16))
    Wih1T = np.ascontiguousarray(Wih1.T.astype(f16))
    Whh1T = np.ascontiguousarray(Whh1.T.astype(f16))
    b0 = np.ascontiguousarray((bih0 + bhh0)[None, :].astype(f16))
    b1 = np.ascontiguousarray((bih1 + bhh1)[None, :].astype(f16))
    owT = np.ascontiguousarray(out_w.T.astype(f16))
    ob = np.ascontiguousarray(out_b[None, :].astype(f16))

    def blocked_T(x):  # [BC,H] -> [128, 8*BC] (k-blocks of columns)
        t = np.ascontiguousarray(x.T)          # [H, BC]
        return np.ascontiguousarray(
            t.reshape(8, 128, BC).transpose(1, 0, 2).reshape(128, 8 * BC)
        ).astype(f16)

    ids = np.asarray(input_ids).reshape(B)
    in_maps = []
    for c in range(NCORES):
        bs = slice(BC * c, BC * (c + 1))
        enc_c = encoder_outputs[bs]                      # [BC, S, H]
        encT = np.ascontiguousarray(
            enc_c.reshape(TOK, H).T.astype(f16))         # [H, TOK] b-major
        encS = np.ascontiguousarray(
            enc_c.transpose(1, 0, 2).reshape(TOK, H).astype(f16))  # s-major
        ohT = np.zeros((V, BC), f16)
        ohT[ids[bs].astype(np.int64), np.arange(BC)] = 1.0
        in_maps.append({
            "encT": encT, "encS": encS, "Umat": U16, "Wmat": W16,
            "VwR": VwR,
            "hT0": blocked_T(hidden[0][bs]),
            "hT1": blocked_T(hidden[1][bs]),
            "onehotT": ohT, "embW": emb16, "oh64": oh64, "I2": I2,
            "I128": I128, "ones1": ones1,
            "Wih0T": Wih0T, "Whh0T": Whh0T, "Wih1T": Wih1T, "Whh1T": Whh1T,
            "bias0": b0, "bias1": b1,
            "cell0": np.ascontiguousarray(cell[0][bs], dtype=np.float32),
            "cell1": np.ascontiguousarray(cell[1][bs], dtype=np.float32),
            "outWT": owT, "outB": ob,
        })
    return in_maps


def kernel(input_ids, hidden, cell, encoder_outputs, emb, U, W, Vw,
           Wih0, Whh0, bih0, bhh0, Wih1, Whh1, bih1, bhh1,
           out_w, out_b, matrix=0, _trace=False):
    if _COMPILED[0] is None:
        _COMPILED[0] = _build()
    nc = _COMPILED[0]
    args = [np.asarray(a) for a in
            (input_ids, hidden, cell, encoder_outputs, emb, U, W, Vw,
             Wih0, Whh0, bih0, bhh0, Wih1, Whh1, bih1, bhh1, out_w, out_b)]
    in_maps = _prep_inputs(*args)
    res = run_bass_kernel_spmd(nc, in_maps, core_ids=list(range(NCORES)),
                               trace=_trace)
    outs = res.results
    logits = np.concatenate([outs[c]["logits"] for c in range(NCORES)], 0)
    h_new = np.stack([
        np.concatenate([outs[c]["h0n"] for c in range(NCORES)], 0),
        np.concatenate([outs[c]["h1n"] for c in range(NCORES)], 0)])
    c_new = np.stack([
        np.concatenate([outs[c]["c0n"] for c in range(NCORES)], 0),
        np.concatenate([outs[c]["c1n"] for c in range(NCORES)], 0)])
    out = logits[:, None, :].astype(np.float32)
    kernel._last_results = res
    if int(np.asarray(matrix)):
        raise NotImplementedError("matrix=1 path not needed (reference uses 0)")
    return (out, h_new.astype(np.float32), c_new.astype(np.float32))


# revision 15
# speedup vs baseline: 1.4163x; 1.0409x over previous
"""Trainium2 Bass kernel for one attention-LSTM decoder step.

dims: B=512, S=64, H=1024, E=256, V=128, L=2, sharded data-parallel over
batch across 8 NeuronCores (64 batches/core). All matmuls run in fp16 with
fp32 PSUM accumulation; elementwise/softmax math in fp32.
"""

import sys

if "/opt/trn_rl_repo" not in sys.path:
    sys.path.insert(0, "/opt/trn_rl_repo")

import numpy as np

import concourse.bacc as bacc
import concourse.mybir as mybir
import concourse.tile as tile
from concourse.bass_utils import run_bass_kernel_spmd

B, S, H, E, V = 512, 64, 1024, 256, 128
NCORES = 8
BC = B // NCORES          # 64 batches per core
TOK = BC * S              # 4096 tokens per core
F = E + H                 # 1280 LSTM input features
G4 = 4 * H                # 4096 gate rows
F16 = mybir.dt.float16
F32 = mybir.dt.float32
AF = mybir.ActivationFunctionType
ALU = mybir.AluOpType
AX = mybir.AxisListType

_COMPILED = [None]


def _build():
    nc = bacc.Bacc("TRN2", target_bir_lowering=False, debug=False,
                   num_devices=NCORES)

    # ---- DRAM I/O ----
    d_encT = nc.dram_tensor("encT", [H, TOK], F16, kind="ExternalInput")
    d_encS = nc.dram_tensor("encS", [TOK, H], F16, kind="ExternalInput")
    d_U = nc.dram_tensor("Umat", [H, H], F16, kind="ExternalInput")
    d_W = nc.dram_tensor("Wmat", [H, H], F16, kind="ExternalInput")
    d_Vw = nc.dram_tensor("VwR", [128, 8], F16, kind="ExternalInput")
    d_hT0 = nc.dram_tensor("hT0", [128, 8 * BC], F16, kind="ExternalInput")
    d_hT1 = nc.dram_tensor("hT1", [128, 8 * BC], F16, kind="ExternalInput")
    d_ohT = nc.dram_tensor("onehotT", [V, BC], F16, kind="ExternalInput")
    d_emb = nc.dram_tensor("embW", [V, E], F16, kind="ExternalInput")
    d_oh64 = nc.dram_tensor("oh64", [BC, TOK], F16, kind="ExternalInput")
    d_I2 = nc.dram_tensor("I2", [128, 64], F32, kind="ExternalInput")
    d_I128 = nc.dram_tensor("I128", [128, 128], F16, kind="ExternalInput")
    d_ones1 = nc.dram_tensor("ones1", [1, BC], F16, kind="ExternalInput")
    d_wi0 = nc.dram_tensor("Wih0T", [F, G4], F16, kind="ExternalInput")
    d_wh0 = nc.dram_tensor("Whh0T", [H, G4], F16, kind="ExternalInput")
    d_wi1 = nc.dram_tensor("Wih1T", [H, G4], F16, kind="ExternalInput")
    d_wh1 = nc.dram_tensor("Whh1T", [H, G4], F16, kind="ExternalInput")
    d_b0 = nc.dram_tensor("bias0", [1, G4], F16, kind="ExternalInput")
    d_b1 = nc.dram_tensor("bias1", [1, G4], F16, kind="ExternalInput")
    d_c0 = nc.dram_tensor("cell0", [BC, H], F32, kind="ExternalInput")
    d_c1 = nc.dram_tensor("cell1", [BC, H], F32, kind="ExternalInput")
    d_owT = nc.dram_tensor("outWT", [H, V], F16, kind="ExternalInput")
    d_ob = nc.dram_tensor("outB", [1, V], F16, kind="ExternalInput")

    d_logits = nc.dram_tensor("logits", [BC, V], F32, kind="ExternalOutput")
    d_h0n = nc.dram_tensor("h0n", [BC, H], F32, kind="ExternalOutput")
    d_h1n = nc.dram_tensor("h1n", [BC, H], F32, kind="ExternalOutput")
    d_c0n = nc.dram_tensor("c0n", [BC, H], F32, kind="ExternalOutput")
    d_c1n = nc.dram_tensor("c1n", [BC, H], F32, kind="ExternalOutput")

    with tile.TileContext(nc) as tc:
        with (
            tc.tile_pool(name="const", bufs=1) as cpool,
            tc.tile_pool(name="stream", bufs=3) as stp,
            tc.tile_pool(name="tanh", bufs=2) as thp,
            tc.tile_pool(name="wls", bufs=3) as wpool,
            tc.tile_pool(name="mid", bufs=1) as mid,
            tc.tile_pool(name="psA", bufs=2, space="PSUM") as psA,
            tc.tile_pool(name="psE", bufs=2, space="PSUM") as psE,
            tc.tile_pool(name="psS", bufs=2, space="PSUM") as psS,
            tc.tile_pool(name="psT", bufs=2, space="PSUM") as psT,
        ):
            # ---------- resident constants ----------
            vw_sb = cpool.tile([128, 8], F16, tag="vw")
            nc.sync.dma_start(vw_sb[:], d_Vw[:])
            hT0_sb = cpool.tile([128, 8 * BC], F16, tag="hT0")
            nc.sync.dma_start(hT0_sb[:], d_hT0[:])
            hT1_sb = cpool.tile([128, 8 * BC], F16, tag="hT1")
            nc.sync.dma_start(hT1_sb[:], d_hT1[:])
            ohT_sb = cpool.tile([V, BC], F16, tag="ohT")
            nc.sync.dma_start(ohT_sb[:], d_ohT[:])
            emb_sb = cpool.tile([V, E], F16, tag="emb")
            nc.sync.dma_start(emb_sb[:], d_emb[:])
            ones1_sb = cpool.tile([1, BC], F16, tag="ones1")
            nc.sync.dma_start(ones1_sb[:], d_ones1[:])

            # ---------- phase A: t2 = h_top @ W  -> [BC, H] f16 ----------
            t2_sb = mid.tile([BC, H], F16, tag="t2")
            for n2 in range(2):
                ps = psS.tile([BC, 512], F32, tag="ps64", name="t2ps")
                wsl = stp.tile([128, 4096], F16, tag="big", name="wsl")
                nc.sync.dma_start(
                    wsl[:].rearrange("p (k c) -> p k c", c=512),
                    d_W.rearrange("(k p) h -> k p h", p=128)
                       [:, :, 512 * n2:512 * (n2 + 1)]
                       .transpose([1, 0, 2]))
                for k in range(8):
                    nc.tensor.matmul(
                        ps[:], hT1_sb[:, 64 * k:64 * (k + 1)],
                        wsl[:, 512 * k:512 * (k + 1)],
                        start=(k == 0), stop=(k == 7))
                nc.vector.tensor_copy(t2_sb[:, 512 * n2:512 * (n2 + 1)], ps[:])

            U_sb = [cpool.tile([128, H], F16, tag=f"U{k}", name=f"U{k}") for k in range(8)]
            for k in range(8):
                nc.sync.dma_start(U_sb[k][:], d_U[128 * k:128 * (k + 1), :])
            # ---------- phase B: embedded^T -> xT blocks 0..1 ----------
            xT_sb = mid.tile([128, 10 * BC], F16, tag="xT")
            for et in range(2):
                ps = psT.tile([128, BC], F32, tag="pT", name="embps")
                nc.tensor.matmul(ps[:], emb_sb[:, 128 * et:128 * (et + 1)],
                                 ohT_sb[:], start=True, stop=True)
                nc.vector.tensor_copy(xT_sb[:, BC * et:BC * (et + 1)], ps[:])

            I128_sb = cpool.tile([128, 128], F16, tag="I128")
            nc.sync.dma_start(I128_sb[:], d_I128[:])
            c0_sb = cpool.tile([BC, H], F32, tag="c0")
            nc.sync.dma_start(c0_sb[:], d_c0[:])
            c1_sb = cpool.tile([BC, H], F32, tag="c1")
            nc.sync.dma_start(c1_sb[:], d_c1[:])
            owT_sb = [cpool.tile([128, V], F16, tag=f"ow{k}", name=f"ow{k}") for k in range(8)]
            for k in range(8):
                nc.sync.dma_start(owT_sb[k][:], d_owT[128 * k:128 * (k + 1), :])
            ob_sb = cpool.tile([1, V], F16, tag="ob")
            nc.sync.dma_start(ob_sb[:], d_ob[:])
            # ---------- phase C: t1 + t2 -> tanh -> scores -> softmax -> ct ---
            # token tile n holds ALL 64 s-positions of batches 8n..8n+8, so
            # each tile's softmax + attention-context can be computed inline
            # against the already-resident encT tile (no second enc stream).
            ctT_sb = mid.tile([128, 512], F32, tag="ctT")  # [h-blk k][8n+b] cols
            for n in range(8):          # token tiles (512 tokens, 8 batches)
                et = stp.tile([128, 4096], F16, tag="big", name="et")
                nc.sync.dma_start(
                    et[:].rearrange("p (k c) -> p k c", c=512),
                    d_encT.rearrange("(k p) t -> k p t", p=128)
                          [:, :, 512 * n:512 * (n + 1)]
                          .transpose([1, 0, 2]))
                oh64_t = stp.tile([BC, 512], F16, tag="oh", name="oh64_t")
                nc.sync.dma_start(oh64_t[:], d_oh64[:, 512 * n:512 * (n + 1)])
                pe = psE.tile([1, 512], F32, tag="eps")
                for m in range(8):      # output-H tiles
                    pt = psA.tile([128, 512], F32, tag="t1ps")
                    for k in range(8):
                        nc.tensor.matmul(
                            pt[:], U_sb[k][:, 128 * m:128 * (m + 1)],
                            et[:, 512 * k:512 * (k + 1)],
                            start=(k == 0), stop=False)
                    # inject t2 broadcast over s:  lhsT=[64b,128h'] rhs=[64b,512tok]
                    nc.tensor.matmul(
                        pt[:], t2_sb[:, 128 * m:128 * (m + 1)],
                        oh64_t[:], start=False, stop=True)
                    th = thp.tile([128, 512], F16, tag="tanh")
                    nc.scalar.activation(th[:], pt[:], AF.Tanh)
                    nc.tensor.matmul(pe[:], vw_sb[:, m:m + 1], th[:],
                                     start=(m == 0), stop=(m == 7))
                # --- inline softmax over s for batches 8n..8n+8 ---
                er = mid.tile([1, 512], F32, tag="er", name="er", bufs=2)
                nc.vector.tensor_copy(er[:], pe[:])
                eb = mid.tile([8, S], F32, tag="eb", name="eb", bufs=2)
                nc.gpsimd.dma_start(
                    eb[:], er[0:1, :].rearrange("p (b s) -> p b s", b=8))
                mx = mid.tile([8, 1], F32, tag="mx", name="mx", bufs=2)
                nc.vector.tensor_reduce(mx[:], eb[:], axis=AX.X, op=ALU.max)
                negmx = mid.tile([8, 1], F32, tag="negmx", name="negmx", bufs=2)
                nc.vector.tensor_scalar_mul(negmx[:], mx[:], -1.0)
                pb = mid.tile([8, S], F32, tag="pb", name="pb", bufs=2)
                nc.scalar.activation(pb[:], eb[:], AF.Exp, bias=negmx[:])
                sm = mid.tile([8, 1], F32, tag="sm", name="sm", bufs=2)
                nc.vector.tensor_reduce(sm[:], pb[:], axis=AX.X, op=ALU.add)
                rinv = mid.tile([8, 1], F32, tag="rinv", name="rinv", bufs=2)
                nc.vector.reciprocal(rinv[:], sm[:])
                ab = mid.tile([8, S], F16, tag="ab", name="ab", bufs=2)
                nc.vector.tensor_scalar_mul(ab[:], pb[:], rinv[:])
                ar = mid.tile([1, 512], F16, tag="ar", name="ar", bufs=2)
                nc.gpsimd.dma_start(
                    ar[0:1, :].rearrange("p (b s) -> p b s", b=8), ab[:])
                af = mid.tile([128, 512], F16, tag="af", name="af", bufs=2)
                nc.gpsimd.partition_broadcast(af[:], ar[:])
                # --- ct^T columns for these 8 batches, per h-block k ---
                for k in range(8):
                    pr = thp.tile([128, 512], F16, tag="prod", name="pr", bufs=2)
                    nc.vector.tensor_tensor(
                        pr[:], et[:, 512 * k:512 * (k + 1)], af[:], ALU.mult)
                    nc.vector.tensor_reduce(
                        ctT_sb[:, 64 * k + 8 * n:64 * k + 8 * n + 8],
                        pr[:].rearrange("p (b s) -> p b s", b=8),
                        axis=AX.X, op=ALU.add)
            # ct^T blocked [h-blk][b] is exactly xT blocks 2..9
            nc.vector.tensor_copy(xT_sb[:, 2 * BC:10 * BC], ctT_sb[:])

            # ---------- phases G/H: two LSTM layers ----------
            def lstm_layer(xT, n_xk, wxd, whd, hT, bias_d, c_in,
                           d_hout, d_cout, hTout, lname):
                """xT: SBUF [128, n_xk*64] input^T blocks; whd/wxd DRAM weights;
                hT: SBUF [128, 8*64] prev-h^T blocks; returns nothing."""
                gates = mid.tile([BC, G4], F16, tag="gates", name="gates")
                bt = mid.tile([1, G4], F16, tag="btile", name="bt")
                nc.sync.dma_start(bt[:], bias_d[:])
                wx_v = wxd.rearrange("(k p) g -> k p g", p=128)
                wh_v = whd.rearrange("(k p) g -> k p g", p=128)
                for n in range(8):
                    # batched loads: all k-slices of this 512-gate column
                    wx = wpool.tile([128, n_xk * 512], F16, tag="wx", name="wx")
                    nc.scalar.dma_start(
                        wx[:].rearrange("p (k c) -> p k c", c=512),
                        wx_v[:, :, 512 * n:512 * (n + 1)].transpose([1, 0, 2]))
                    wh = wpool.tile([128, 8 * 512], F16, tag="wh", name="wh")
                    nc.sync.dma_start(
                        wh[:].rearrange("p (k c) -> p k c", c=512),
                        wh_v[:, :, 512 * n:512 * (n + 1)].transpose([1, 0, 2]))
                    ps = psS.tile([BC, 512], F32, tag="ps64", name="gps")
                    for k in range(n_xk):
                        nc.tensor.matmul(ps[:], xT[:, 64 * k:64 * (k + 1)],
                                         wx[:, 512 * k:512 * (k + 1)],
                                         start=(k == 0), stop=False)
                    for k in range(8):
                        nc.tensor.matmul(ps[:], hT[:, 64 * k:64 * (k + 1)],
                                         wh[:, 512 * k:512 * (k + 1)],
                                         start=False, stop=False)
                    nc.tensor.matmul(ps[:], ones1_sb[:],
                                     bt[:, 512 * n:512 * (n + 1)],
                                     start=False, stop=True)
                    func = AF.Tanh if n in (4, 5) else AF.Sigmoid
                    nc.scalar.activation(gates[:, 512 * n:512 * (n + 1)],
                                         ps[:], func)
                # c2 = sig_f*c + sig_i*tanh_g ; h2 = sig_o*tanh(c2)
                tmp = mid.tile([BC, H], F32, tag="lstm_tmp", name="tmp")
                nc.vector.tensor_tensor(tmp[:], gates[:, 0:H],
                                        gates[:, 2 * H:3 * H], ALU.mult)
                c2 = mid.tile([BC, H], F32, tag="c2t", name="c2")
                nc.vector.tensor_tensor(c2[:], gates[:, H:2 * H], c_in[:],
                                        ALU.mult)
                nc.vector.tensor_tensor(c2[:], c2[:], tmp[:], ALU.add)
                nc.sync.dma_start(d_cout[:], c2[:])
                tc2 = mid.tile([BC, H], F32, tag="lstm_tmp", name="tc2")
                nc.scalar.activation(tc2[:], c2[:], AF.Tanh)
                h2 = mid.tile([BC, H], F32, tag="h2t", name="h2")
                nc.vector.tensor_tensor(h2[:], gates[:, 3 * H:4 * H], tc2[:],
                                        ALU.mult)
                nc.sync.dma_start(d_hout[:], h2[:])
                h2f = mid.tile([BC, H], F16, tag="lstm_h2f", name="h2f")
                nc.vector.tensor_copy(h2f[:], h2[:])
                for j in range(8):
                    pt = psT.tile([128, BC], F16, tag="pT", name="trps")
                    nc.tensor.transpose(pt[:], h2f[:, 128 * j:128 * (j + 1)],
                                        I128_sb[0:64, 0:64])
                    nc.vector.tensor_copy(hTout[:, BC * j:BC * (j + 1)], pt[:])

            h0T_sb = mid.tile([128, 8 * BC], F16, tag="h0T")
            lstm_layer(xT_sb, 10, d_wi0, d_wh0, hT0_sb, d_b0, c0_sb,
                       d_h0n, d_c0n, h0T_sb, "l0")
            h1T_sb = mid.tile([128, 8 * BC], F16, tag="h1T")
            lstm_layer(h0T_sb, 8, d_wi1, d_wh1, hT1_sb, d_b1, c1_sb,
                       d_h1n, d_c1n, h1T_sb, "l1")

            # ---------- phase I: logits ----------
            pl = psS.tile([BC, V], F32, tag="ps64", name="lps")
            for k in range(8):
                nc.tensor.matmul(pl[:], h1T_sb[:, 64 * k:64 * (k + 1)],
                                 owT_sb[k][:], start=(k == 0), stop=False)
            nc.tensor.matmul(pl[:], ones1_sb[:], ob_sb[:],
                             start=False, stop=True)
            lo = mid.tile([BC, V], F32, tag="lo")
            nc.vector.tensor_copy(lo[:], pl[:])
            nc.sync.dma_start(d_logits[:], lo[:])

    nc.compile()
    return nc


def _prep_inputs(input_ids, hidden, cell, encoder_outputs, emb, U, W, Vw,
                 Wih0, Whh0, bih0, bhh0, Wih1, Whh1, bih1, bhh1,
                 out_w, out_b):
    f16 = np.float16
    # shared across cores
    U16 = np.ascontiguousarray(U.astype(f16))
    W16 = np.ascontiguousarray(W.astype(f16))
    VwR = np.ascontiguousarray(Vw.reshape(8, 128).T.astype(f16))  # [128,8]
    emb16 = np.ascontiguousarray(emb.astype(f16))
    oh64 = np.zeros((BC, TOK), f16)
    for b in range(BC):
        oh64[b, 64 * b:64 * (b + 1)] = 1.0
    I2 = np.zeros((128, 64), np.float32)
    I2[np.arange(128), np.arange(128) % 64] = 1.0
    I128 = np.eye(128, dtype=f16)
    ones1 = np.ones((1, BC), f16)
    Wih0T = np.ascontiguousarray(Wih0.T.astype(f16))
    Whh0T = np.ascontiguousarray(Whh0.T.astype(f16))
    Wih1T = np.ascontiguousarray(Wih1.T.astype(f16))
    Whh1T = np.ascontiguousarray(Whh1.T.astype(f16))
    b0 = np.ascontiguousarray((bih0 + bhh0)[None, :].astype(f16))
    b1 = np.ascontiguousarray((bih1 + bhh1)[None, :].astype(f16))
    owT = np.ascontiguousarray(out_w.T.astype(f16))
    ob = np.ascontiguousarray(out_b[None, :].astype(f16))

    def blocked_T(x):  # [BC,H] -> [128, 8*BC] (k-blocks of columns)
        t = np.ascontiguousarray(x.T)          # [H, BC]
        return np.ascontiguousarray(
            t.reshape(8, 128, BC).transpose(1, 0, 2).reshape(128, 8 * BC)
        ).astype(f16)

    ids = np.asarray(input_ids).reshape(B)
    in_maps = []
    for c in range(NCORES):
        bs = slice(BC * c, BC * (c + 1))
        enc_c = encoder_outputs[bs]                      # [BC, S, H]
        encT = np.ascontiguousarray(
            enc_c.reshape(TOK, H).T.astype(f16))         # [H, TOK] b-major
        encS = np.ascontiguousarray(
            enc_c.transpose(1, 0, 2).reshape(TOK, H).astype(f16))  # s-major
        ohT = np.zeros((V, BC), f16)
        ohT[ids[bs].astype(np.int64), np.arange(BC)] = 1.0
        in_maps.append({
            "encT": encT, "encS": encS, "Umat": U16, "Wmat": W16,
            "VwR": VwR,
            "hT0": blocked_T(hidden[0][bs]),
            "hT1": blocked_T(hidden[1][bs]),
            "onehotT": ohT, "embW": emb16, "oh64": oh64, "I2": I2,
            "I128": I128, "ones1": ones1,
            "Wih0T": Wih0T, "Whh0T": Whh0T, "Wih1T": Wih1T, "Whh1T": Whh1T,
            "bias0": b0, "bias1": b1,
            "cell0": np.ascontiguousarray(cell[0][bs], dtype=np.float32),
            "cell1": np.ascontiguousarray(cell[1][bs], dtype=np.float32),
            "outWT": owT, "outB": ob,
        })
    return in_maps


def kernel(input_ids, hidden, cell, encoder_outputs, emb, U, W, Vw,
           Wih0, Whh0, bih0, bhh0, Wih1, Whh1, bih1, bhh1,
           out_w, out_b, matrix=0, _trace=False):
    if _COMPILED[0] is None:
        _COMPILED[0] = _build()
    nc = _COMPILED[0]
    args = [np.asarray(a) for a in
            (input_ids, hidden, cell, encoder_outputs, emb, U, W, Vw,
             Wih0, Whh0, bih0, bhh0, Wih1, Whh1, bih1, bhh1, out_w, out_b)]
    in_maps = _prep_inputs(*args)
    res = run_bass_kernel_spmd(nc, in_maps, core_ids=list(range(NCORES)),
                               trace=_trace)
    outs = res.results
    logits = np.concatenate([outs[c]["logits"] for c in range(NCORES)], 0)
    h_new = np.stack([
        np.concatenate([outs[c]["h0n"] for c in range(NCORES)], 0),
        np.concatenate([outs[c]["h1n"] for c in range(NCORES)], 0)])
    c_new = np.stack([
        np.concatenate([outs[c]["c0n"] for c in range(NCORES)], 0),
        np.concatenate([outs[c]["c1n"] for c in range(NCORES)], 0)])
    out = logits[:, None, :].astype(np.float32)
    kernel._last_results = res
    if int(np.asarray(matrix)):
        raise NotImplementedError("matrix=1 path not needed (reference uses 0)")
    return (out, h_new.astype(np.float32), c_new.astype(np.float32))
